# revision 12
# baseline (speedup 1.0000x reference)
"""Block-sparse attention (SageAttention-style mean-similarity top-k) on 8 TRN2 NeuronCores.

Sharding: 16 heads tensor-parallel across 8 cores (2 heads/core).
  - qkv weight column-sharded per core; block selection + block-sparse
    attention fully local per head
  - proj weight row-sharded: each core computes the full-shape PARTIAL product
    (+ bias on core 0 only) in fp16; the host unshard step sums the partials.

v5:
  - fp16 end-to-end (selection still f32-from-fp16-x sums: verified exact).
  - host pre-chunks x/weights/output into SBUF-partition-major layout so each
    DMA needs 128 large descriptors instead of 1024 small ones (the v4 x load
    was descriptor-generation-bound at ~6us per 1MB chunk).
  - phase A: q/k only in the chunk loop (PE ~3.4us/chunk ~= DMA), v matmuls
    deferred to overlap the selection chain; phase B starts ~15us earlier.
  - phase B: AV trails scores by 2 iterations (vg gathers are late because v
    is deferred); o_ps and pj share a 2-slot PSUM tag so o ping-pongs and AV
    never waits on the previous norm; obounce on the sync queue (the v4
    scalar-queue DGE config serialized exp behind the DVE norm).
"""

import os
import sys

for _p in ("/opt/trn_rl_repo", "/root/.axon_site/_ro/trn_rl_repo"):
    if os.path.isdir(_p) and _p not in sys.path:
        sys.path.insert(0, _p)

import numpy as np

import concourse.bass as bass
import concourse.bacc as bacc
import concourse.tile as tile
import concourse.mybir as mybir
from concourse.bass_utils import run_bass_kernel_spmd
from concourse.library_config import ap_gather as ap_gather_lib

# problem constants
N = 4096          # sequence length
C = 1024          # model dim
H = 16            # heads
D = 64            # head dim
BLK = 128         # block size
NB = N // BLK     # 32 blocks
TOPK = 16         # int(0.5 * NB)
NCORES = 8
HPC = H // NCORES  # 2 heads per core
SCALE = D ** -0.5  # 0.125

F32 = mybir.dt.float32
F16 = mybir.dt.float16
I16 = mybir.dt.int16
U32 = mybir.dt.uint32

CHQ = 4            # query blocks per projection chunk
CHT = CHQ * BLK    # 512 tokens per chunk
LAG = 2            # AV trails scores by this many iterations

_CACHE = {}


def _build():
    nc = bacc.Bacc("TRN2", target_bir_lowering=False, debug=False,
                   num_devices=NCORES)

    KC = C // 128  # 8 contraction tiles

    # pre-chunked host layouts: [..., p, a, m] so each partition's data is
    # contiguous (128 big DMA descriptors instead of 1024 small ones)
    xT5 = nc.dram_tensor("xT5", [8, 128, KC, 512], F16, kind="ExternalInput")
    wqkvT = nc.dram_tensor("wqkvT", [128, KC, 384], F16, kind="ExternalInput")
    wqkT32 = nc.dram_tensor("wqkT32", [128, KC, 256], F32, kind="ExternalInput")
    projWT = nc.dram_tensor("projWT", [2 * D, C], F16, kind="ExternalInput")
    projb = nc.dram_tensor("projb", [128, KC], F32, kind="ExternalInput")
    ident64 = nc.dram_tensor("ident64", [64, 64], F32, kind="ExternalInput")
    erep = nc.dram_tensor("erep", [16, 128], F32, kind="ExternalInput")
    out5 = nc.dram_tensor("out", [8, 128, KC, 512], F16, kind="ExternalOutput")

    obounce = nc.dram_tensor("obounce", [N, 2 * D], F16)

    with tile.TileContext(nc) as tc:
        nc.gpsimd.load_library(ap_gather_lib)

        with tc.tile_pool(name="persist", bufs=1) as pp:
            qT = pp.tile([128, N], F16)
            kT = pp.tile([128, NB, BLK], F16)   # contiguous == [128, N]
            v0 = pp.tile([128, NB, 66], F16)
            v1 = pp.tile([128, NB, 66], F16)
            nc.vector.memset(v0[:, :, 64:66], 0.0)
            nc.vector.memset(v1[:, :, 64:66], 0.0)
            nc.vector.memset(v0[:, :, 64:65], 1.0)
            nc.vector.memset(v1[:, :, 64:65], 1.0)

            xm = pp.tile([128, KC, NB], F32)
            xall = pp.tile([128, KC, N], F16)   # x resident for deferred v

            # ---- weights: wqkv on sync (needed first), the rest on gpsimd ----
            wqkv_h = pp.tile([128, KC, 384], F16)
            nc.sync.dma_start(wqkv_h[:], wqkvT.ap())
            wqk_f32 = pp.tile([128, KC, 256], F32)
            nc.gpsimd.dma_start(wqk_f32[:], wqkT32.ap())
            projW_h = pp.tile([128, C], F16)          # [c_local, j]
            nc.gpsimd.dma_start(projW_h[:], projWT.ap())
            projb_sb = pp.tile([128, KC], F32)
            nc.gpsimd.dma_start(projb_sb[:], projb.ap())
            id64 = pp.tile([64, 64], F32)
            nc.gpsimd.dma_start(id64[:], ident64.ap())
            erep_sb = pp.tile([16, 128], F32)
            nc.gpsimd.dma_start(erep_sb[:], erep.ap())

            kidx = pp.tile([128, NB], I16)
            vidx0 = pp.tile([128, NB], I16)
            vidx1 = pp.tile([128, NB], I16)

            # ---- phase A: x chunks -> block sums + q/k only ----
            with tc.tile_pool(name="qkps", bufs=3, space="PSUM") as qp, \
                 tc.tile_pool(name="vps", bufs=2, space="PSUM") as vp, \
                 tc.tile_pool(name="selps", bufs=1, space="PSUM") as sp, \
                 tc.tile_pool(name="selsb", bufs=2) as sb:
                for nch in range(8):
                    lo, hi = nch * 512, (nch + 1) * 512
                    nc.sync.dma_start(xall[:, :, lo:hi], xT5.ap()[nch])
                    for kc in range(KC):
                        nc.vector.tensor_reduce(
                            xm[:, kc, nch * 4:(nch + 1) * 4],
                            xall[:, kc, lo:hi].rearrange("p (b t) -> p b t", t=BLK),
                            axis=mybir.AxisListType.X, op=mybir.AluOpType.add)
                    for mt in (0, 1):
                        ps = qp.tile([128, 512], F32, tag="qk")
                        for kc in range(KC):
                            nc.tensor.matmul(
                                ps[:], lhsT=wqkv_h[:, kc, mt * 128:(mt + 1) * 128],
                                rhs=xall[:, kc, lo:hi],
                                start=(kc == 0), stop=(kc == KC - 1))
                        if mt == 0:
                            nc.scalar.copy(qT[:, lo:hi], ps[:])
                        else:
                            nc.scalar.copy(
                                kT[:].rearrange("p a b -> p (a b)")[:, lo:hi],
                                ps[:])

                # ---- selection (f32) ----
                qm_ps = sp.tile([128, NB], F32, tag="qm")
                km_ps = sp.tile([128, NB], F32, tag="km")
                for kc in range(KC):
                    nc.tensor.matmul(qm_ps[:], lhsT=wqk_f32[:, kc, 0:128],
                                     rhs=xm[:, kc, :], start=(kc == 0), stop=(kc == KC - 1))
                for kc in range(KC):
                    nc.tensor.matmul(km_ps[:], lhsT=wqk_f32[:, kc, 128:256],
                                     rhs=xm[:, kc, :], start=(kc == 0), stop=(kc == KC - 1))
                qm_sb = sb.tile([128, NB], F32, tag="qm")
                km_sb = sb.tile([128, NB], F32, tag="km")
                nc.scalar.copy(qm_sb[:], qm_ps[:])
                nc.scalar.copy(km_sb[:], km_ps[:])

                sim_ps = sp.tile([64, NB], F32, tag="sel")
                for h in range(HPC):
                    nc.tensor.matmul(sim_ps[h * 32:(h + 1) * 32, :],
                                     lhsT=qm_sb[h * 64:(h + 1) * 64, :],
                                     rhs=km_sb[h * 64:(h + 1) * 64, :],
                                     start=True, stop=True)
                sim2 = sb.tile([64, NB], F32, tag="sim2")
                nc.vector.tensor_copy(sim2[:], sim_ps[:])

                vals0 = sb.tile([64, 8], F32, tag="v0")
                idx0 = sb.tile([64, 8], U32, tag="i0")
                pun = sb.tile([64, NB], F32, tag="pun")
                vals1 = sb.tile([64, 8], F32, tag="v1")
                idx1 = sb.tile([64, 8], U32, tag="i1")
                nc.vector.max(vals0[:], sim2[:])
                nc.vector.max_index(idx0[:], vals0[:], sim2[:])
                nc.vector.match_replace(out=pun[:], in_to_replace=vals0[:],
                                        in_values=sim2[:], imm_value=-1e30)
                nc.vector.max(vals1[:], pun[:])
                nc.vector.max_index(idx1[:], vals1[:], pun[:])

                idxf = sb.tile([64, TOPK], F32, tag="idxf")
                nc.vector.tensor_copy(idxf[:, 0:8], idx0[:])
                nc.vector.tensor_copy(idxf[:, 8:16], idx1[:])

                selT_ps = sp.tile([TOPK, 64], F32, tag="sel", name="selT_ps")
                nc.tensor.transpose(selT_ps[:], idxf[:], id64[:])
                selT = sb.tile([TOPK, 64], F32, tag="selTsb")
                nc.vector.tensor_copy(selT[:], selT_ps[:])

                rep_ps = sp.tile([128, 64], F32, tag="sel", name="rep_ps")
                nc.tensor.matmul(rep_ps[:], lhsT=erep_sb[:], rhs=selT[:],
                                 start=True, stop=True)
                nc.vector.tensor_copy(kidx[0:64, :], rep_ps[0:64, 0:32])
                nc.vector.tensor_copy(kidx[64:128, :], rep_ps[64:128, 32:64])
                nc.vector.tensor_copy(vidx0[:], rep_ps[:, 0:32])
                nc.vector.tensor_copy(vidx1[:], rep_ps[:, 32:64])

                # ---- deferred v matmuls (overlap the selection DVE chain) ----
                for nt in range(NB):
                    psv = vp.tile([128, 128], F32, tag="v")
                    for kc in range(KC):
                        nc.tensor.matmul(
                            psv[:],
                            lhsT=xall[:, kc, nt * 128:(nt + 1) * 128],
                            rhs=wqkv_h[:, kc, 256:384],
                            start=(kc == 0), stop=(kc == KC - 1))
                    nc.vector.tensor_copy(v0[:, nt, 0:64], psv[:, 0:64])
                    nc.vector.tensor_copy(v1[:, nt, 0:64], psv[:, 64:128])

            # ---- main loop ----
            with tc.tile_pool(name="kgp", bufs=3) as kgp, \
                 tc.tile_pool(name="vgp", bufs=5) as vgp, \
                 tc.tile_pool(name="escore", bufs=16) as ep, \
                 tc.tile_pool(name="sps", bufs=3, space="PSUM") as spp, \
                 tc.tile_pool(name="mix", bufs=2, space="PSUM") as mpp, \
                 tc.tile_pool(name="onp", bufs=3) as onp, \
                 tc.tile_pool(name="otsb", bufs=2) as otp, \
                 tc.tile_pool(name="posb", bufs=2) as pop:

                kgs = {}
                vgs = {}
                escores = {}
                o_pss = {}
                ot_tiles = {}
                po_tiles = {}
                proj_q = []

                def emit_kg(qb):
                    kg = kgp.tile([128, TOPK, BLK], F16, tag="kg", name=f"kg_{qb}")
                    nc.gpsimd.ap_gather(kg[:], kT[:], kidx[:, qb:qb + 1],
                                        channels=128, num_elems=NB, d=BLK, num_idxs=TOPK)
                    kgs[qb] = kg

                def emit_vg(qb):
                    vg0 = vgp.tile([128, TOPK, 66], F16, tag="vg0", name=f"vg0_{qb}")
                    nc.gpsimd.ap_gather(vg0[:], v0[:], vidx0[:, qb:qb + 1],
                                        channels=128, num_elems=NB, d=66, num_idxs=TOPK)
                    vg1 = vgp.tile([128, TOPK, 66], F16, tag="vg1", name=f"vg1_{qb}")
                    nc.gpsimd.ap_gather(vg1[:], v1[:], vidx1[:, qb:qb + 1],
                                        channels=128, num_elems=NB, d=66, num_idxs=TOPK)
                    vgs[qb] = (vg0, vg1)

                def emit_scores_half(qb, half):
                    kg = kgs[qb]
                    qcol = slice(qb * BLK, (qb + 1) * BLK)
                    s0 = spp.tile([128, 1024], F32, tag="s", name=f"s0_{qb}_{half}")
                    s1 = spp.tile([128, 1024], F32, tag="s", name=f"s1_{qb}_{half}")
                    for jj in range(8):
                        j = half * 8 + jj
                        nc.tensor.matmul(s0[:, jj * 128:(jj + 1) * 128],
                                         lhsT=kg[0:64, j, :], rhs=qT[0:64, qcol],
                                         start=True, stop=True)
                        nc.tensor.matmul(s1[:, jj * 128:(jj + 1) * 128],
                                         lhsT=kg[64:128, j, :], rhs=qT[64:128, qcol],
                                         start=True, stop=True)
                    e0 = ep.tile([128, 1024], F16, tag="e", name=f"e0_{qb}_{half}")
                    e1 = ep.tile([128, 1024], F16, tag="e", name=f"e1_{qb}_{half}")
                    nc.scalar.activation(e0[:], s0[:],
                                         mybir.ActivationFunctionType.Exp, scale=SCALE)
                    nc.scalar.activation(e1[:], s1[:],
                                         mybir.ActivationFunctionType.Exp, scale=SCALE)
                    if half == 0:
                        escores[qb] = [[e0, None], [e1, None]]
                    else:
                        escores[qb][0][1] = e0
                        escores[qb][1][1] = e1

                def emit_av(qb, h):
                    etiles = escores[qb]
                    if h == 0:
                        o_pss[qb] = mpp.tile([128, 2, 66], F32, tag="mix",
                                             name=f"o_{qb}")
                    o_ps = o_pss[qb]
                    vg = vgs[qb][h]
                    for j in range(TOPK):
                        nc.tensor.matmul(o_ps[:, h, 0:65],
                                         lhsT=etiles[h][j // 8][:, (j % 8) * 128:(j % 8 + 1) * 128],
                                         rhs=vg[:, j, 0:65],
                                         start=(j == 0), stop=(j == TOPK - 1))

                def emit_norm(qb):
                    o_ps = o_pss.pop(qb)
                    onorm = onp.tile([128, 2 * D], F16, tag="onorm", name=f"on_{qb}")
                    for h in (0, 1):
                        rec = onp.tile([128, 1], F32, tag="rec", name=f"r_{qb}_{h}")
                        nc.vector.reciprocal(rec[:], o_ps[:, h, 64:65])
                        nc.vector.tensor_scalar(onorm[:, h * D:(h + 1) * D],
                                                o_ps[:, h, 0:D], rec[:], None,
                                                op0=mybir.AluOpType.mult)
                    nc.sync.dma_start(obounce.ap()[qb * BLK:(qb + 1) * BLK, :],
                                      onorm[:])
                    del escores[qb]
                    del kgs[qb]
                    del vgs[qb]
                    if qb % CHQ == CHQ - 1:
                        c = qb // CHQ
                        ot = otp.tile([128, CHT], F16, tag="ot", name=f"ot_{c}")
                        nc.sync.dma_start_transpose(
                            ot[:], obounce.ap()[c * CHT:(c + 1) * CHT, :])
                        ot_tiles[c] = ot
                        po_tiles[c] = pop.tile([128, KC, CHT], F16, tag="po",
                                               name=f"po_{c}")
                        for m in range(KC):
                            proj_q.append((c, m))

                def emit_proj_some(k):
                    for _ in range(k):
                        if not proj_q:
                            return
                        c, m = proj_q.pop(0)
                        pj = mpp.tile([128, CHT], F32, tag="mix", name=f"pj_{c}_{m}")
                        nc.tensor.matmul(pj[:],
                                         lhsT=projW_h[:, m * 128:(m + 1) * 128],
                                         rhs=ot_tiles[c][:], start=True, stop=True)
                        nc.vector.tensor_scalar(po_tiles[c][:, m, :], pj[:],
                                                projb_sb[:, m:m + 1], None,
                                                op0=mybir.AluOpType.add)
                        if m == KC - 1:
                            nc.sync.dma_start(out5.ap()[c], po_tiles.pop(c)[:])
                            ot_tiles.pop(c)

                # gather prologue: kg first (scores need them immediately),
                # vg after (v finishes during the first scores iterations)
                for i in range(LAG + 1):
                    emit_kg(i)
                for i in range(LAG + 1):
                    emit_vg(i)

                for qb in range(NB):
                    if qb + LAG + 1 < NB:
                        emit_kg(qb + LAG + 1)
                        emit_vg(qb + LAG + 1)
                    emit_proj_some(1)
                    emit_scores_half(qb, 0)
                    if qb >= LAG:
                        emit_av(qb - LAG, 0)
                    emit_proj_some(1)
                    emit_scores_half(qb, 1)
                    if qb >= LAG:
                        emit_av(qb - LAG, 1)
                        emit_norm(qb - LAG)
                # epilogue
                for qb in range(NB - LAG, NB):
                    emit_av(qb, 0)
                    emit_av(qb, 1)
                    emit_norm(qb)
                    emit_proj_some(2)
                emit_proj_some(len(proj_q))

    nc.compile()
    return nc


def _prep_inputs(x, qkv_w, proj_w, proj_b):
    x = np.asarray(x, dtype=np.float32)
    qkv_w = np.asarray(qkv_w, dtype=np.float32)
    proj_w = np.asarray(proj_w, dtype=np.float32)
    proj_b = np.asarray(proj_b, dtype=np.float32)

    xT = x[0].T.astype(np.float16)                         # [C, N]
    # [nch, p, a, m] partition-major chunks
    # xT.reshape(KC_a, 128_p, 8_nch, 512_m) -> want [nch, p, a, m]
    xT5 = np.ascontiguousarray(
        xT.reshape(8, 128, 8, 512).transpose(2, 1, 0, 3))
    ident64 = np.eye(64, dtype=np.float32)
    erep = (np.arange(128)[None, :] % 16 == np.arange(16)[:, None]).astype(np.float32)
    zero_b = np.zeros((128, 8), dtype=np.float32)
    in_maps = []
    for i in range(NCORES):
        h0 = HPC * i
        rows = []
        for part in range(3):                              # q, k, v row groups
            base = part * C + h0 * D
            rows.append(qkv_w[base:base + HPC * D, :])
        wqkv = np.concatenate(rows, axis=0)                # [384, C]
        wqkvT_np = np.ascontiguousarray(wqkv.T)            # [C, 384]
        # [p, a, m] layout
        wq5 = np.ascontiguousarray(
            wqkvT_np.reshape(8, 128, 384).transpose(1, 0, 2))
        wq5_32 = np.ascontiguousarray(
            wqkvT_np[:, 0:256].reshape(8, 128, 256).transpose(1, 0, 2))
        cslice = slice(i * 2 * D, (i + 1) * 2 * D)
        in_maps.append({
            "xT5": xT5,
            "wqkvT": wq5.astype(np.float16),
            "wqkT32": wq5_32,
            "projWT": np.ascontiguousarray(proj_w[:, cslice].T).astype(np.float16),
            "projb": (np.ascontiguousarray(proj_b.reshape(8, 128).T)
                      if i == 0 else zero_b),
            "ident64": ident64,
            "erep": erep,
        })
    return in_maps


def kernel(x, qkv_w, proj_w, proj_b, _trace=False):
    if "nc" not in _CACHE:
        _CACHE["nc"] = _build()
    nc = _CACHE["nc"]
    in_maps = _prep_inputs(x, qkv_w, proj_w, proj_b)
    res = run_bass_kernel_spmd(nc, in_maps, core_ids=list(range(NCORES)),
                               trace=_trace)
    # out5 [8_c, 128_p, KC_a, 512_m] -> out[a*128+p, c*512+m]
    acc = res.results[0]["out"].astype(np.float32)
    for i in range(1, NCORES):
        acc += res.results[i]["out"].astype(np.float32)
    outT = np.ascontiguousarray(
        acc.transpose(2, 1, 0, 3).reshape(C, N))
    out = np.ascontiguousarray(outT.T).reshape(1, N, C).astype(np.float32)
    if _trace:
        _CACHE["last_exec_time_ns"] = res.exec_time_ns
        _CACHE["last_results"] = res
    return out
